# revision 8
# baseline (speedup 1.0000x reference)
"""Bass/Trainium2 kernel for nn_Bilinear (out[b,n,i] = enc[b,n,i,:] @ W @ hidden[b,:] + bias).

Sharding: data-parallel over B. 8 cores, one batch element each.
Per core:
  stage 1 (TensorE): v[j] = sum_k W[j,k] * h[k].  We feed Wt = W.T from the
    host so the contraction dim k sits on SBUF partitions; 16 small matmuls
    (lhsT = h chunk [128,1], rhs = Wt rows [128,512]) accumulate v into PSUM.
  stage 2 (VectorE): stream enc rows as [128, 8, 1024] tiles (4 MiB DMAs) and
    fuse multiply+reduce per 128-row block with tensor_tensor_reduce; the bias
    rides as the reduction's initial value.
Output is written per-core as out[b].T ([128 i, 64 n]); host transposes back.
"""

import os

import numpy as np

B, N, I, H = 8, 64, 128, 1024
P = 128
NI = N * I  # 8192 rows per core
N_CORES = 8

_NC_CACHE = {}
LAST_RESULTS = None


def _build(ni_rows=NI):
    import concourse.bacc as bacc
    import concourse.mybir as mybir
    import concourse.tile as tile
    from concourse import dve_ops

    f32 = mybir.dt.float32
    CHUNK = 1024 if ni_rows >= 1024 else ni_rows  # rows per DMA
    T = ni_rows // CHUNK  # chunks
    C = CHUNK // P  # 128-row blocks per chunk
    KB = H // P  # k blocks for stage 1

    nc = bacc.Bacc(
        "TRN2",
        target_bir_lowering=False,
        debug=False,
        num_devices=N_CORES,
    )
    enc = nc.declare_dram_parameter("enc", [ni_rows, H], f32, isOutput=False)
    hh = nc.declare_dram_parameter("h", [1, H], f32, isOutput=False)
    wt = nc.declare_dram_parameter("wt", [H, H], f32, isOutput=False)
    bb = nc.declare_dram_parameter("bias", [1, 1], f32, isOutput=False)
    out = nc.declare_dram_parameter("out_t", [P, ni_rows // P], f32, isOutput=True)

    with tile.TileContext(nc) as tc:
        with (
            tc.tile_pool(name="const", bufs=1) as const,
            tc.tile_pool(name="epool", bufs=3) as epool,
            tc.tile_pool(name="vpsum", bufs=2, space="PSUM") as vpsum,
            tc.tile_pool(name="dram", bufs=1, space="DRAM") as dram,
        ):
            # ---- stage 1: v[j] = sum_k Wt[k,j] h[k] ----
            wt_sb = const.tile([P, KB * H], f32)
            nc.sync.dma_start(
                out=wt_sb[:].rearrange("p (kb j) -> p kb j", kb=KB),
                in_=wt[:, :].rearrange("(kb p) j -> p kb j", p=P),
            )
            h_col = const.tile([P, KB], f32)
            nc.sync.dma_start(
                out=h_col[:],
                in_=hh[:, :].rearrange("a (kb p) -> p (a kb)", p=P),
            )
            bias_col = const.tile([P, 1], f32)
            nc.sync.dma_start(out=bias_col[:], in_=bb[:, :].to_broadcast((P, 1)))

            v_flat = const.tile([1, H], f32)
            for jc in range(H // 512):
                vp = vpsum.tile([1, 512], f32)
                for kb in range(KB):
                    nc.tensor.matmul(
                        vp[:],
                        h_col[:, kb : kb + 1],
                        wt_sb[:, kb * H + jc * 512 : kb * H + jc * 512 + 512],
                        start=(kb == 0),
                        stop=(kb == KB - 1),
                    )
                nc.any.tensor_copy(v_flat[:, jc * 512 : (jc + 1) * 512], vp[:])
            # partition-broadcast v: bounce through DRAM, then stride-0 DMA read
            v_dram = dram.tile([1, H], f32)
            nc.sync.dma_start(out=v_dram[:], in_=v_flat[:])
            v_rep = const.tile([P, H], f32)
            nc.gpsimd.dma_start(out=v_rep[:], in_=v_dram[:].to_broadcast((P, H)))

            # ---- stage 2: out[col*128+p] = bias + sum_j enc[row, j] * v[j] ----
            out_sb = const.tile([P, ni_rows // P], f32)
            dummy = const.tile([P, 1], f32)
            enc_r = enc[:, :].rearrange("(t c p) j -> t p c j", t=T, c=C)
            for t in range(T):
                e_tile = epool.tile([P, C, H], f32)
                nc.sync.dma_start(out=e_tile[:], in_=enc_r[t])
                for c in range(C):
                    col = t * C + c
                    # fused (e * v) multiply + add-reduce on DVE; bias seeds
                    # the accumulator (custom-DVE op — the plain ISA
                    # TENSOR_TENSOR_REDUCE opcode faults on this runtime)
                    nc.vector._custom_dve(
                        dve_ops.TENSOR_TENSOR_REDUCE,
                        out=dummy[:].broadcast_to((P, H)),
                        in0=e_tile[:, c],
                        in1=v_rep[:],
                        s0=bias_col[:],
                        s1=1.0,
                        accum_out=out_sb[:, col : col + 1],
                    )
            nc.sync.dma_start(out=out[:, :], in_=out_sb[:])
    nc.compile()
    return nc


def _get_nc():
    if "nc" not in _NC_CACHE:
        _NC_CACHE["nc"] = _build()
    return _NC_CACHE["nc"]


def kernel(hidden=None, encoder_hiddens=None, input_lengths=None, W=None, b=None):
    global LAST_RESULTS
    from concourse.bass_utils import run_bass_kernel_spmd

    hidden = np.asarray(hidden, dtype=np.float32)
    enc = np.asarray(encoder_hiddens, dtype=np.float32)
    W_ = np.asarray(W, dtype=np.float32)
    b_ = np.asarray(b, dtype=np.float32).reshape(1, 1)
    wt = np.ascontiguousarray(W_.T)

    nc = _get_nc()
    in_maps = []
    for core in range(N_CORES):
        in_maps.append(
            {
                "enc": np.ascontiguousarray(enc[core].reshape(NI, H)),
                "h": np.ascontiguousarray(hidden[core : core + 1, :]),
                "wt": wt,
                "bias": b_,
            }
        )
    res = run_bass_kernel_spmd(nc, in_maps, core_ids=list(range(N_CORES)))
    LAST_RESULTS = res
    out = np.stack([res.results[i]["out_t"].T for i in range(N_CORES)])
    return np.ascontiguousarray(out.astype(np.float32))


# revision 11
# speedup vs baseline: 1.0183x; 1.0183x over previous
"""Bass/Trainium2 kernel for nn_Bilinear (out[b,n,i] = enc[b,n,i,:] @ W @ hidden[b,:] + bias).

Sharding: data-parallel over B. 8 cores, one batch element each.
Per core:
  stage 1 (TensorE): v[j] = sum_k W[j,k] * h[k].  Host feeds Wt = W.T so the
    contraction dim k sits on SBUF partitions; Wt streams in as 8 chunked
    0.5 MiB DMAs and 16 small matmuls (lhsT = h chunk [128,1], rhs = Wt rows
    [128,512]) pipeline behind them, accumulating v into PSUM.  v is then
    bounced through DRAM and partition-broadcast to [128, 1024].
  stage 2 (VectorE + ScalarE): stream enc rows as [128, 4, 1024] tiles
    (2 MiB DMAs, issued after the Wt chunks so the HWDGE FIFO finishes Wt
    first); per 128-row block, DVE does the elementwise multiply against the
    broadcast v and ScalarE reduces it with an accumulate-Copy activation.
    Bias is added once at the end on the [128, 64] result.
Output is written per-core as out[b].T ([128 i, 64 n]); host transposes back.
"""

import numpy as np

B, N, I, H = 8, 64, 128, 1024
P = 128
NI = N * I  # 8192 rows per core
N_CORES = 8

_NC_CACHE = {}
LAST_RESULTS = None


def _build(ni_rows=NI, chunk_rows=512, ebufs=6):
    import concourse.bacc as bacc
    import concourse.mybir as mybir
    import concourse.tile as tile

    f32 = mybir.dt.float32
    CHUNK = min(chunk_rows, ni_rows)  # rows per E DMA
    T = ni_rows // CHUNK  # chunks
    C = CHUNK // P  # 128-row blocks per chunk
    KB = H // P  # k blocks for stage 1

    nc = bacc.Bacc(
        "TRN2",
        target_bir_lowering=False,
        debug=False,
        num_devices=N_CORES,
    )
    enc = nc.declare_dram_parameter("enc", [ni_rows, H], f32, isOutput=False)
    hh = nc.declare_dram_parameter("h", [P, KB], f32, isOutput=False)
    wt = nc.declare_dram_parameter("wt", [H, H], f32, isOutput=False)
    bb = nc.declare_dram_parameter("bias", [1, 1], f32, isOutput=False)
    out = nc.declare_dram_parameter("out_t", [P, ni_rows // P], f32, isOutput=True)

    with tile.TileContext(nc) as tc:
        with (
            tc.tile_pool(name="const", bufs=1) as const,
            tc.tile_pool(name="epool", bufs=ebufs) as epool,
            tc.tile_pool(name="ppool", bufs=3) as ppool,
            tc.tile_pool(name="vpsum", bufs=2, space="PSUM") as vpsum,
            tc.tile_pool(name="dram", bufs=1, space="DRAM") as dram,
        ):
            # ---- stage 1: v[j] = sum_k Wt[k,j] h[k] ----
            # Wt in KB chunks so matmuls start as soon as the first arrives,
            # and so the HWDGE FIFO drains Wt before the E stream.
            wt_sb = const.tile([P, KB * H], f32)
            for kb in range(KB):
                nc.sync.dma_start(
                    out=wt_sb[:, kb * H : (kb + 1) * H],
                    in_=wt[kb * P : (kb + 1) * P, :],
                )
            h_col = const.tile([P, KB], f32)
            nc.sync.dma_start(out=h_col[:], in_=hh[:, :])
            bias_col = const.tile([P, 1], f32)
            nc.sync.dma_start(out=bias_col[:], in_=bb[:, :].to_broadcast((P, 1)))

            v_flat = const.tile([1, H], f32)
            vps = [
                vpsum.tile([1, 512], f32, name=f"vp{jc}", tag=f"vp{jc}")
                for jc in range(H // 512)
            ]
            for kb in range(KB):
                for jc in range(H // 512):
                    nc.tensor.matmul(
                        vps[jc][:],
                        h_col[:, kb : kb + 1],
                        wt_sb[:, kb * H + jc * 512 : kb * H + jc * 512 + 512],
                        start=(kb == 0),
                        stop=(kb == KB - 1),
                    )
            for jc in range(H // 512):
                nc.scalar.activation(
                    v_flat[:, jc * 512 : (jc + 1) * 512],
                    vps[jc][:],
                    mybir.ActivationFunctionType.Copy,
                )
            # partition-broadcast v: bounce through DRAM, then stride-0 DMA read
            v_dram = dram.tile([1, H], f32)
            nc.sync.dma_start(out=v_dram[:], in_=v_flat[:])
            v_rep = const.tile([P, H], f32)
            nc.gpsimd.dma_start(out=v_rep[:], in_=v_dram[:].to_broadcast((P, H)))

            # ---- stage 2: out[col*128+p] = sum_j enc[row, j] * v[j] ----
            out_sb = const.tile([P, ni_rows // P], f32)
            enc_r = enc[:, :].rearrange("(t c p) j -> t p c j", t=T, c=C)
            for t in range(T):
                e_tile = epool.tile([P, C, H], f32)
                nc.sync.dma_start(out=e_tile[:], in_=enc_r[t])
                for c in range(C):
                    col = t * C + c
                    prod = ppool.tile([P, H], f32)
                    nc.vector.tensor_mul(prod[:], e_tile[:, c], v_rep[:])
                    nc.scalar.activation(
                        prod[:],
                        prod[:],
                        mybir.ActivationFunctionType.Copy,
                        accum_out=out_sb[:, col : col + 1],
                    )
            # bias once over the whole [128, ni/128] result
            nc.vector.tensor_scalar_add(out_sb[:], out_sb[:], bias_col[:])
            nc.sync.dma_start(out=out[:, :], in_=out_sb[:])
    nc.compile()
    return nc


def _get_nc():
    if "nc" not in _NC_CACHE:
        _NC_CACHE["nc"] = _build()
    return _NC_CACHE["nc"]


def kernel(hidden=None, encoder_hiddens=None, input_lengths=None, W=None, b=None):
    global LAST_RESULTS
    from concourse.bass_utils import run_bass_kernel_spmd

    hidden = np.asarray(hidden, dtype=np.float32)
    enc = np.asarray(encoder_hiddens, dtype=np.float32)
    W_ = np.asarray(W, dtype=np.float32)
    b_ = np.asarray(b, dtype=np.float32).reshape(1, 1)
    wt = np.ascontiguousarray(W_.T)

    nc = _get_nc()
    KB = H // P
    in_maps = []
    for core in range(N_CORES):
        in_maps.append(
            {
                "enc": np.ascontiguousarray(enc[core].reshape(NI, H)),
                "h": np.ascontiguousarray(hidden[core].reshape(KB, P).T),
                "wt": wt,
                "bias": b_,
            }
        )
    res = run_bass_kernel_spmd(nc, in_maps, core_ids=list(range(N_CORES)))
    LAST_RESULTS = res
    out = np.stack([res.results[i]["out_t"].T for i in range(N_CORES)])
    return np.ascontiguousarray(out.astype(np.float32))


# revision 15
# speedup vs baseline: 1.0839x; 1.0644x over previous
"""Bass/Trainium2 kernel for nn_Bilinear (out[b,n,i] = enc[b,n,i,:] @ W @ hidden[b,:] + bias).

Sharding: data-parallel over B. 8 cores, one batch element each.
Per core:
  stage 1 (TensorE): v[j] = sum_k W[j,k] * h[k].  Host feeds Wt = W.T so the
    contraction dim k sits on SBUF partitions; Wt streams in as 8 chunked
    0.5 MiB DMAs and 16 small matmuls (lhsT = h chunk [128,1], rhs = Wt rows
    [128,512]) pipeline behind them, accumulating v into PSUM.  v is then
    bounced through DRAM and partition-broadcast to [128, 1024].
  stage 2 (VectorE + ScalarE): stream enc rows as [128, 4, 1024] tiles
    (2 MiB DMAs, issued after the Wt chunks so the HWDGE FIFO finishes Wt
    first); per 128-row block, DVE does the elementwise multiply against the
    broadcast v and ScalarE reduces it with an accumulate-Copy activation.
    Bias is added once at the end on the [128, 64] result.
Output is written per-core as out[b].T ([128 i, 64 n]); host transposes back.
"""

import numpy as np

B, N, I, H = 8, 64, 128, 1024
P = 128
NI = N * I  # 8192 rows per core
N_CORES = 8

_NC_CACHE = {}
LAST_RESULTS = None


def _build(ni_rows=NI, chunk_rows=512, ebufs=6):
    import concourse.bacc as bacc
    import concourse.mybir as mybir
    import concourse.tile as tile
    from concourse import dve_ops

    f32 = mybir.dt.float32
    CHUNK = min(chunk_rows, ni_rows)  # rows per E DMA
    T = ni_rows // CHUNK  # chunks
    C = CHUNK // P  # 128-row blocks per chunk
    KB = H // P  # k blocks for stage 1

    nc = bacc.Bacc(
        "TRN2",
        target_bir_lowering=False,
        debug=False,
        num_devices=N_CORES,
    )
    enc = nc.declare_dram_parameter("enc", [ni_rows, H], f32, isOutput=False)
    hh = nc.declare_dram_parameter("h", [P, KB], f32, isOutput=False)
    wt = nc.declare_dram_parameter("wt", [H, H], f32, isOutput=False)
    bb = nc.declare_dram_parameter("bias", [1, 1], f32, isOutput=False)
    out = nc.declare_dram_parameter("out_t", [P, ni_rows // P], f32, isOutput=True)

    with tile.TileContext(nc) as tc:
        with (
            tc.tile_pool(name="const", bufs=1) as const,
            tc.tile_pool(name="epool", bufs=ebufs) as epool,
            tc.tile_pool(name="ppool", bufs=3) as ppool,
            tc.tile_pool(name="vpsum", bufs=2, space="PSUM") as vpsum,
            tc.tile_pool(name="dram", bufs=1, space="DRAM") as dram,
        ):
            # ---- stage 1: v[j] = sum_k Wt[k,j] h[k] ----
            # Wt in KB chunks so matmuls start as soon as the first arrives,
            # and so the HWDGE FIFO drains Wt before the E stream.
            wt_sbs = []
            for kb in range(KB):
                wt_kb = const.tile([P, H], f32, name=f"wt{kb}", tag=f"wt{kb}")
                nc.sync.dma_start(out=wt_kb[:], in_=wt[kb * P : (kb + 1) * P, :])
                wt_sbs.append(wt_kb)
            h_col = const.tile([P, KB], f32)
            nc.sync.dma_start(out=h_col[:], in_=hh[:, :])
            bias_col = const.tile([P, 1], f32)
            nc.sync.dma_start(out=bias_col[:], in_=bb[:, :].to_broadcast((P, 1)))

            v_flat = const.tile([1, H], f32)
            vps = [
                vpsum.tile([1, 512], f32, name=f"vp{jc}", tag=f"vp{jc}")
                for jc in range(H // 512)
            ]
            for kb in range(KB):
                for jc in range(H // 512):
                    nc.tensor.matmul(
                        vps[jc][:],
                        h_col[:, kb : kb + 1],
                        wt_sbs[kb][:, jc * 512 : (jc + 1) * 512],
                        start=(kb == 0),
                        stop=(kb == KB - 1),
                    )
            for jc in range(H // 512):
                nc.scalar.activation(
                    v_flat[:, jc * 512 : (jc + 1) * 512],
                    vps[jc][:],
                    mybir.ActivationFunctionType.Copy,
                )
            # partition-broadcast v: bounce through DRAM, then stride-0 DMA read
            v_dram = dram.tile([1, H], f32)
            nc.sync.dma_start(out=v_dram[:], in_=v_flat[:])
            v_rep = const.tile([P, H], f32)
            nc.gpsimd.dma_start(out=v_rep[:], in_=v_dram[:].to_broadcast((P, H)))

            # ---- stage 2: out[col*128+p] = sum_j enc[row, j] * v[j] ----
            # Per chunk, 1 of 4 blocks uses the fused all-DVE TTR and the
            # other 3 use DVE-mul + ScalarE accumulate-Copy, balancing the
            # two engines (~5 us/chunk each) under the ~5.5 us/chunk DMA.
            out_sb = const.tile([P, ni_rows // P], f32)
            dummy = const.tile([P, 1], f32)
            enc_r = enc[:, :].rearrange("(t c p) j -> t p c j", t=T, c=C)
            for t in range(T):
                e_tile = epool.tile([P, C, H], f32)
                nc.sync.dma_start(out=e_tile[:], in_=enc_r[t])
                for c in range(C):
                    col = t * C + c
                    if c % 4 == 0:
                        nc.vector._custom_dve(
                            dve_ops.TENSOR_TENSOR_REDUCE,
                            out=dummy[:].broadcast_to((P, H)),
                            in0=e_tile[:, c],
                            in1=v_rep[:],
                            s0=0.0,
                            s1=1.0,
                            accum_out=out_sb[:, col : col + 1],
                        )
                    else:
                        prod = ppool.tile([P, H], f32)
                        nc.vector.tensor_mul(prod[:], e_tile[:, c], v_rep[:])
                        nc.scalar.activation(
                            prod[:],
                            prod[:],
                            mybir.ActivationFunctionType.Copy,
                            accum_out=out_sb[:, col : col + 1],
                        )
            # bias once over the whole [128, ni/128] result
            nc.vector.tensor_scalar_add(out_sb[:], out_sb[:], bias_col[:])
            nc.sync.dma_start(out=out[:, :], in_=out_sb[:])
    nc.compile()
    return nc


def _get_nc():
    if "nc" not in _NC_CACHE:
        _NC_CACHE["nc"] = _build()
    return _NC_CACHE["nc"]


def kernel(hidden=None, encoder_hiddens=None, input_lengths=None, W=None, b=None):
    global LAST_RESULTS
    from concourse.bass_utils import run_bass_kernel_spmd

    hidden = np.asarray(hidden, dtype=np.float32)
    enc = np.asarray(encoder_hiddens, dtype=np.float32)
    W_ = np.asarray(W, dtype=np.float32)
    b_ = np.asarray(b, dtype=np.float32).reshape(1, 1)
    wt = np.ascontiguousarray(W_.T)

    nc = _get_nc()
    KB = H // P
    in_maps = []
    for core in range(N_CORES):
        in_maps.append(
            {
                "enc": np.ascontiguousarray(enc[core].reshape(NI, H)),
                "h": np.ascontiguousarray(hidden[core].reshape(KB, P).T),
                "wt": wt,
                "bias": b_,
            }
        )
    res = run_bass_kernel_spmd(nc, in_maps, core_ids=list(range(N_CORES)))
    LAST_RESULTS = res
    out = np.stack([res.results[i]["out_t"].T for i in range(N_CORES)])
    return np.ascontiguousarray(out.astype(np.float32))


# revision 16
# speedup vs baseline: 1.2363x; 1.1406x over previous
"""Bass/Trainium2 kernel for nn_Bilinear (out[b,n,i] = enc[b,n,i,:] @ W @ hidden[b,:] + bias).

Sharding: data-parallel over B. 8 cores, one batch element each.
Per core:
  stage 1 (TensorE): v[j] = sum_k W[j,k] * h[k].  Host feeds Wt = W.T so the
    contraction dim k sits on SBUF partitions; Wt streams in as 8 chunked
    0.5 MiB DMAs and 16 small matmuls (lhsT = h chunk [128,1], rhs = Wt rows
    [128,512]) pipeline behind them, accumulating v into PSUM.  v is then
    bounced through DRAM and partition-broadcast to [128, 1024].
  stage 2 (VectorE + ScalarE): stream enc rows as [128, 4, 1024] tiles
    (2 MiB DMAs, issued after the Wt chunks so the HWDGE FIFO finishes Wt
    first); per 128-row block, DVE does the elementwise multiply against the
    broadcast v and ScalarE reduces it with an accumulate-Copy activation.
    Bias is added once at the end on the [128, 64] result.
Output is written per-core as out[b].T ([128 i, 64 n]); host transposes back.
"""

import numpy as np

B, N, I, H = 8, 64, 128, 1024
P = 128
NI = N * I  # 8192 rows per core
N_CORES = 8

_NC_CACHE = {}
LAST_RESULTS = None


def _build(ni_rows=NI, chunk_rows=512, ebufs=7):
    import concourse.bacc as bacc
    import concourse.mybir as mybir
    import concourse.tile as tile
    from concourse import dve_ops

    f32 = mybir.dt.float32
    CHUNK = min(chunk_rows, ni_rows)  # rows per E DMA
    T = ni_rows // CHUNK  # chunks
    C = CHUNK // P  # 128-row blocks per chunk
    KB = H // P  # k blocks for stage 1

    nc = bacc.Bacc(
        "TRN2",
        target_bir_lowering=False,
        debug=False,
        num_devices=N_CORES,
    )
    enc = nc.declare_dram_parameter("enc", [ni_rows, H], f32, isOutput=False)
    hh = nc.declare_dram_parameter("h", [P, KB], f32, isOutput=False)
    wt = nc.declare_dram_parameter("wt", [H, H], f32, isOutput=False)
    bb = nc.declare_dram_parameter("bias", [1, 1], f32, isOutput=False)
    out = nc.declare_dram_parameter("out_t", [P, ni_rows // P], f32, isOutput=True)

    with tile.TileContext(nc) as tc:
        with (
            tc.tile_pool(name="const", bufs=1) as const,
            tc.tile_pool(name="epool", bufs=ebufs) as epool,
            tc.tile_pool(name="ppool", bufs=3) as ppool,
            tc.tile_pool(name="vpsum", bufs=1, space="PSUM") as vpsum,
        ):
            # ---- stage 1: v[j] = sum_k Wt[k,j] h[k] ----
            # h/bias first (tiny) so matmuls only ever wait on Wt chunks;
            # Wt in KB chunks so matmuls pipeline behind their arrival.
            h_col = const.tile([P, KB], f32)
            nc.sync.dma_start(out=h_col[:], in_=hh[:, :])
            bias_col = const.tile([P, 1], f32)
            nc.sync.dma_start(out=bias_col[:], in_=bb[:, :].to_broadcast((P, 1)))
            wt_sbs = []
            for kb in range(KB):
                wt_kb = const.tile([P, H], f32, name=f"wt{kb}", tag=f"wt{kb}")
                nc.sync.dma_start(out=wt_kb[:], in_=wt[kb * P : (kb + 1) * P, :])
                wt_sbs.append(wt_kb)
            ones = const.tile([1, P], f32)
            nc.vector.memset(ones[:], 1.0)

            v_flat = const.tile([1, H], f32)
            vps = [
                vpsum.tile([1, 512], f32, name=f"vp{jc}", tag=f"vp{jc}")
                for jc in range(H // 512)
            ]
            for kb in range(KB):
                for jc in range(H // 512):
                    nc.tensor.matmul(
                        vps[jc][:],
                        h_col[:, kb : kb + 1],
                        wt_sbs[kb][:, jc * 512 : (jc + 1) * 512],
                        start=(kb == 0),
                        stop=(kb == KB - 1),
                    )
            for jc in range(H // 512):
                nc.scalar.activation(
                    v_flat[:, jc * 512 : (jc + 1) * 512],
                    vps[jc][:],
                    mybir.ActivationFunctionType.Copy,
                )
            # partition-broadcast v on the PE: ones[1,P].T @ v[1,512] -> [P,512]
            v_rep = const.tile([P, H], f32)
            for jc in range(H // 512):
                bc = vpsum.tile([P, 512], f32, name=f"bc{jc}", tag=f"bc{jc}")
                nc.tensor.matmul(
                    bc[:],
                    ones[:],
                    v_flat[:, jc * 512 : (jc + 1) * 512],
                    start=True,
                    stop=True,
                )
                nc.scalar.activation(
                    v_rep[:, jc * 512 : (jc + 1) * 512],
                    bc[:],
                    mybir.ActivationFunctionType.Copy,
                )

            # ---- stage 2: out[col*128+p] = sum_j enc[row, j] * v[j] ----
            # Per chunk, 1 of 4 blocks uses the fused all-DVE TTR and the
            # other 3 use DVE-mul + ScalarE accumulate-Copy, balancing the
            # two engines (~5 us/chunk each) under the ~5.5 us/chunk DMA.
            out_sb = const.tile([P, ni_rows // P], f32)
            dummy = const.tile([P, 1], f32)
            enc_r = enc[:, :].rearrange("(t c p) j -> t p c j", t=T, c=C)
            for t in range(T):
                e_tile = epool.tile([P, C, H], f32)
                nc.sync.dma_start(out=e_tile[:], in_=enc_r[t])
                for c in range(C):
                    col = t * C + c
                    if c % 4 == 0:
                        nc.vector._custom_dve(
                            dve_ops.TENSOR_TENSOR_REDUCE,
                            out=dummy[:].broadcast_to((P, H)),
                            in0=e_tile[:, c],
                            in1=v_rep[:],
                            s0=0.0,
                            s1=1.0,
                            accum_out=out_sb[:, col : col + 1],
                        )
                    else:
                        prod = ppool.tile([P, H], f32)
                        nc.vector.tensor_mul(prod[:], e_tile[:, c], v_rep[:])
                        nc.scalar.activation(
                            prod[:],
                            prod[:],
                            mybir.ActivationFunctionType.Copy,
                            accum_out=out_sb[:, col : col + 1],
                        )
            # bias once over the whole [128, ni/128] result
            nc.vector.tensor_scalar_add(out_sb[:], out_sb[:], bias_col[:])
            nc.sync.dma_start(out=out[:, :], in_=out_sb[:])
    nc.compile()
    return nc


def _get_nc():
    if "nc" not in _NC_CACHE:
        _NC_CACHE["nc"] = _build()
    return _NC_CACHE["nc"]


def kernel(hidden=None, encoder_hiddens=None, input_lengths=None, W=None, b=None):
    global LAST_RESULTS
    from concourse.bass_utils import run_bass_kernel_spmd

    hidden = np.asarray(hidden, dtype=np.float32)
    enc = np.asarray(encoder_hiddens, dtype=np.float32)
    W_ = np.asarray(W, dtype=np.float32)
    b_ = np.asarray(b, dtype=np.float32).reshape(1, 1)
    wt = np.ascontiguousarray(W_.T)

    nc = _get_nc()
    KB = H // P
    in_maps = []
    for core in range(N_CORES):
        in_maps.append(
            {
                "enc": np.ascontiguousarray(enc[core].reshape(NI, H)),
                "h": np.ascontiguousarray(hidden[core].reshape(KB, P).T),
                "wt": wt,
                "bias": b_,
            }
        )
    res = run_bass_kernel_spmd(nc, in_maps, core_ids=list(range(N_CORES)))
    LAST_RESULTS = res
    out = np.stack([res.results[i]["out_t"].T for i in range(N_CORES)])
    return np.ascontiguousarray(out.astype(np.float32))
